# revision 12
# baseline (speedup 1.0000x reference)
"""Trainium2 Bass kernel for CharOffsetRoPEAttention.

Full-input contract: kernel(**inputs) takes the complete tensors
(x [4,2048,1024], wq/wk/wv/wo [1024,1024], position_ids [4,2048]) and
returns the full output [4,2048,1024].

Sharding: batch x head-group over 8 cores. Core c handles batch c//2 and
heads [8*(c%2), 8*(c%2)+8). q/k/v weights are split column-wise (by head),
wo row-wise; each core computes a partial output for its batch that the
host sums pairwise.

Device dataflow per core (all layouts chosen so the contraction dim sits
on SBUF partitions):
  xT [1024,2048]  (host-pre-transposed x[b])
  qT/kT = (wT.T @ xT) in [dim, t] layout, RoPE applied in-place on chip
  v in [t, dim] layout, with a ones-column per head (softmax denominator
  rides the attn@v matmul for free)
  scores sT[j,i] per head via row-tiled matmuls (two heads share the PE
  array, K=64 each), exp on ACT straight out of PSUM,
  outT[hd+1, i] accumulated over j in PSUM, normalized by the ones-row,
  then out = attn_outT.T @ woT accumulated over the 512 local dims.

RoPE uses a host-side row permutation of wq/wk (per head: evens then
odds) so the rotation is two [128,512] elementwise muls plus four
[32,512] add/subs per chunk instead of strided-pair gymnastics.
"""

import os
from contextlib import ExitStack

import numpy as np

import concourse.bass as bass
import concourse.mybir as mybir
import concourse.tile as tile
from concourse import bacc
from concourse.bass_utils import run_bass_kernel_spmd

B, T, D, H, HD = 4, 2048, 1024, 16, 64
NCORES = 8
HG = 2            # head groups (cores per batch)
HLOC = H // HG    # 8 heads per core
DLOC = HLOC * HD  # 512 local dims per core
KT = D // 128     # 8 k-tiles for the projections
THETA = 10000.0

# matmul operand dtype: "f32" (exact, 4 cyc/row), "f32r" (~tf32, 1 cyc/row),
# "bf16" (1 cyc/row, lowest precision, halves DMA+SBUF)
MM_DT_NAME = os.environ.get("KERNEL_MM_DT", "f32r")

F32 = mybir.dt.float32
EXP = mybir.ActivationFunctionType.Exp


def _mm_dt():
    return {
        "f32": mybir.dt.float32,
        "f32r": mybir.dt.float32r,
        "bf16": mybir.dt.bfloat16,
    }[MM_DT_NAME]


def _dram_in(nc, name, shape):
    """Declare a DRAM input in the wire dtype matching MM_DT."""
    if MM_DT_NAME == "bf16":
        return nc.declare_dram_parameter(name, shape, mybir.dt.bfloat16, isOutput=False)
    return nc.declare_dram_parameter(name, shape, F32, isOutput=False)


def _ld(ap):
    """DRAM-side AP for loading into an MM_DT tile."""
    if MM_DT_NAME == "f32r":
        return ap.bitcast(mybir.dt.float32r)
    return ap


def build_program(loop_n: int = 1):
    """Build and finalize the per-core SPMD Bass program.

    loop_n > 1 wraps the whole body in a device-side For_i — used only for
    timing (difference between loop_n=N and 1 isolates per-iteration time).
    """
    DT = _mm_dt()
    nc = bacc.Bacc()

    xT_d = _dram_in(nc, "xT", [D, T])
    wqT_d = _dram_in(nc, "wqT", [D, DLOC])
    wkT_d = _dram_in(nc, "wkT", [D, DLOC])
    wvT_d = _dram_in(nc, "wvT", [D, DLOC])
    woT_d = _dram_in(nc, "woT", [DLOC, D])
    cos_d = _dram_in(nc, "cosT", [128, T])
    sin_d = _dram_in(nc, "sinT", [128, T])
    out_d = nc.declare_dram_parameter("out", [T, D], F32, isOutput=True)

    with tile.TileContext(nc) as tc, ExitStack() as top:
        if loop_n > 1:
            top.enter_context(tc.For_i(0, loop_n, 1))
        # ---- persistent tiles (live across phases) ----
        pool_qk = top.enter_context(tc.tile_pool(name="qk", bufs=1))
        pool_v = top.enter_context(tc.tile_pool(name="vp", bufs=1))
        qT_t = [pool_qk.tile([128, T], DT, name=f"qT{m}") for m in range(4)]
        kT_t = [pool_qk.tile([128, T], DT, name=f"kT{m}") for m in range(4)]
        v_t = [pool_v.tile([128, HLOC, HD + 1], DT, name=f"v{i}") for i in range(16)]
        pool_on = top.enter_context(tc.tile_pool(name="on", bufs=1))
        ones8 = pool_on.tile([128, HLOC], F32, name="ones8")
        nc.gpsimd.memset(ones8[:], 1.0)
        for i in range(16):
            # softmax-denominator ones column per head
            nc.vector.tensor_copy(v_t[i][:, :, HD : HD + 1], ones8[:].unsqueeze(2))

        # ================= phase A: projections + RoPE =================
        with ExitStack() as phA:
            pool_cs = phA.enter_context(tc.tile_pool(name="cs", bufs=1))
            pool_w = phA.enter_context(tc.tile_pool(name="wt", bufs=1))
            pool_x = phA.enter_context(tc.tile_pool(name="xsl", bufs=10))
            pool_rt = phA.enter_context(tc.tile_pool(name="rt", bufs=4))
            pool_psA = phA.enter_context(tc.tile_pool(name="psA", bufs=4, space="PSUM"))

            cos_sb = pool_cs.tile([128, T], DT, name="cos_sb")
            sin_sb = pool_cs.tile([128, T], DT, name="sin_sb")
            nc.sync.dma_start(out=cos_sb[:], in_=_ld(cos_d[:]))
            nc.sync.dma_start(out=sin_sb[:], in_=_ld(sin_d[:]))

            w_tiles = {}
            for nm, dram in (("q", wqT_d), ("k", wkT_d), ("v", wvT_d)):
                w_tiles[nm] = [
                    pool_w.tile([128, DLOC], DT, name=f"w{nm}{k}") for k in range(KT)
                ]
                for k in range(KT):
                    nc.sync.dma_start(
                        out=w_tiles[nm][k][:],
                        in_=_ld(dram[k * 128 : (k + 1) * 128, :]),
                    )

            for nb in range(4):
                ns = slice(nb * 512, (nb + 1) * 512)
                x_sl = []
                for k in range(KT):
                    xs = pool_x.tile([128, 512], DT, name="x_sl", tag="x_sl")
                    nc.sync.dma_start(
                        out=xs[:], in_=_ld(xT_d[k * 128 : (k + 1) * 128, ns])
                    )
                    x_sl.append(xs)

                # v projection: out [t, dim] — lhsT = xT slices, rhs = wvT
                for tt in range(4):
                    t_idx = nb * 4 + tt
                    pv = pool_psA.tile([128, 512], F32, name="pv", tag="psA")
                    for k in range(KT):
                        nc.tensor.matmul(
                            pv[:],
                            lhsT=x_sl[k][:, tt * 128 : (tt + 1) * 128],
                            rhs=w_tiles["v"][k][:],
                            start=(k == 0),
                            stop=(k == KT - 1),
                        )
                    nc.scalar.copy(
                        out=v_t[t_idx][:, :, 0:HD],
                        in_=pv[:].rearrange("p (h e) -> p h e", h=HLOC),
                    )

                # q/k projections: out [dim, t] — lhsT = w slices, rhs = xT
                for nm, dst in (("q", qT_t), ("k", kT_t)):
                    for m in range(4):
                        pq = pool_psA.tile([128, 512], F32, name="pq", tag="psA")
                        for k in range(KT):
                            nc.tensor.matmul(
                                pq[:],
                                lhsT=w_tiles[nm][k][:, m * 128 : (m + 1) * 128],
                                rhs=x_sl[k][:],
                                start=(k == 0),
                                stop=(k == KT - 1),
                            )
                        nc.scalar.copy(out=dst[m][:, ns], in_=pq[:])
                        # RoPE in place: rows per 64-block are [r(32); i(32)].
                        # q_rot = q*cos + swap32(q)*sin_signed, where sinT rows
                        # carry [+sin; -sin; +sin; -sin] so the swap lands with
                        # the right signs. DVE requires equal input base
                        # partitions, so each quarter mul reads aligned inputs
                        # and writes the swapped-out quarter.
                        tcos = pool_rt.tile([128, 512], DT, name="tcos", tag="rt")
                        tsw = pool_rt.tile([128, 512], DT, name="tsw", tag="rt")
                        nc.gpsimd.tensor_mul(tcos[:], dst[m][:, ns], cos_sb[:, ns])
                        for q0 in (0, 32, 64, 96):
                            src = q0 ^ 32
                            nc.vector.tensor_mul(
                                tsw[q0 : q0 + 32, :],
                                dst[m][src : src + 32, ns],
                                sin_sb[src : src + 32, ns],
                            )
                        nc.gpsimd.tensor_add(dst[m][:, ns], tcos[:], tsw[:])

        # ================= phase B: attention =================
        with ExitStack() as phB:
            pool_ao = phB.enter_context(tc.tile_pool(name="ao", bufs=1))
            pool_wo = phB.enter_context(tc.tile_pool(name="wop", bufs=1))

            ao_t = [pool_ao.tile([128, T], DT, name=f"ao{m}") for m in range(4)]
            woT_t = [pool_wo.tile([128, D], DT, name=f"wo{k}") for k in range(4)]
            for k in range(4):
                nc.sync.dma_start(
                    out=woT_t[k][:], in_=_ld(woT_d[k * 128 : (k + 1) * 128, :])
                )

            attn = ExitStack()
            pool_sm = attn.enter_context(tc.tile_pool(name="sm", bufs=4))
            pool_e = attn.enter_context(tc.tile_pool(name="ep", bufs=4))
            pool_bc = attn.enter_context(tc.tile_pool(name="bcp", bufs=2))
            pool_ps = attn.enter_context(tc.tile_pool(name="ps", bufs=2, space="PSUM"))
            pool_po = attn.enter_context(tc.tile_pool(name="po", bufs=2, space="PSUM"))
            ones_f32 = pool_sm.tile([1, 64], F32, name="ones_f32", tag="ones32")
            nc.gpsimd.memset(ones_f32[:], 1.0)
            ones_sb = pool_sm.tile([1, 64], DT, name="ones_sb", tag="ones")
            nc.vector.tensor_copy(ones_sb[:], ones_f32[:])

            for isb in range(2):
                iss = slice(isb * 1024, (isb + 1) * 1024)
                for hp in range(4):
                    qm, km = qT_t[hp], kT_t[hp]
                    oo = {}
                    for half in (0, 64):
                        oo[half] = pool_po.tile([HD + 1, 1024], F32, name="o_ps", tag="po")
                    for j in range(16):
                        js = slice(j * 128, (j + 1) * 128)
                        s_ps = {}
                        for half in (0, 64):
                            sp = pool_ps.tile([128, 1024], F32, name="s_ps", tag="sc")
                            s_ps[half] = sp
                            for ih in range(2):
                                i0 = isb * 1024 + ih * 512
                                nc.tensor.matmul(
                                    sp[:, ih * 512 : (ih + 1) * 512],
                                    lhsT=km[half : half + 64, js],
                                    rhs=qm[half : half + 64, i0 : i0 + 512],
                                    start=True,
                                    stop=True,
                                    tile_position=(half, 0),
                                )
                        e_t = {}
                        for half in (0, 64):
                            et = pool_e.tile([128, 1024], DT, name="e_t", tag="e")
                            nc.scalar.activation(
                                out=et[:], in_=s_ps[half][:], func=EXP, scale=0.125
                            )
                            e_t[half] = et
                        for half in (0, 64):
                            h_local = hp * 2 + (half // 64)
                            for ih in range(2):
                                nc.tensor.matmul(
                                    oo[half][:, ih * 512 : (ih + 1) * 512],
                                    lhsT=v_t[j][:, h_local, :],
                                    rhs=e_t[half][:, ih * 512 : (ih + 1) * 512],
                                    start=(j == 0),
                                    stop=(j == 15),
                                )
                    # normalize by the ones-row sum and evacuate to attn_outT
                    for half in (0, 64):
                        o_ps = oo[half]
                        rz = pool_sm.tile([1, 1024], DT, name="rz", tag="rz")
                        with nc.allow_low_precision("softmax denom reciprocal in MM dtype"):
                            nc.vector.reciprocal(rz[:], o_ps[HD : HD + 1, :])
                        bc_ps = pool_ps.tile([64, 1024], F32, name="bc_ps", tag="sc")
                        for ih in range(2):
                            nc.tensor.matmul(
                                bc_ps[:, ih * 512 : (ih + 1) * 512],
                                lhsT=ones_sb[:],
                                rhs=rz[:, ih * 512 : (ih + 1) * 512],
                                start=True,
                                stop=True,
                            )
                        bc_sb = pool_bc.tile([64, 1024], F32, name="bc_sb", tag="bc")
                        nc.vector.tensor_copy(bc_sb[:], bc_ps[:])
                        nc.vector.tensor_mul(
                            ao_t[hp][half : half + 64, iss], o_ps[0:HD, :], bc_sb[:]
                        )

            attn.close()

            # ================= phase C: output projection =================
            with (
                tc.tile_pool(name="psC", bufs=4, space="PSUM") as pool_psC,
                tc.tile_pool(name="stC", bufs=4) as pool_st,
            ):
                for tt in range(16):
                    for nblk in range(2):
                        po = pool_psC.tile([128, 512], F32, name="po_c", tag="psC")
                        for k in range(4):
                            nc.tensor.matmul(
                                po[:],
                                lhsT=ao_t[k][:, tt * 128 : (tt + 1) * 128],
                                rhs=woT_t[k][:, nblk * 512 : (nblk + 1) * 512],
                                start=(k == 0),
                                stop=(k == 3),
                            )
                        st = pool_st.tile([128, 512], F32, name="st_c", tag="stC")
                        nc.scalar.copy(out=st[:], in_=po[:])
                        nc.sync.dma_start(
                            out=out_d[tt * 128 : (tt + 1) * 128, nblk * 512 : (nblk + 1) * 512],
                            in_=st[:],
                        )

    nc.finalize()
    return nc


def _wire(a):
    if MM_DT_NAME == "bf16":
        import ml_dtypes

        return np.ascontiguousarray(a).astype(ml_dtypes.bfloat16)
    return np.ascontiguousarray(a, dtype=np.float32)


def prep_in_maps(x, wq, wk, wv, wo, position_ids):
    """Host-side sharding + layout prep. Pure numpy, mirrors reference math
    for the trig tables (f32 throughout, like the jax reference)."""
    x = np.asarray(x, dtype=np.float32)
    wq = np.asarray(wq, dtype=np.float32)
    wk = np.asarray(wk, dtype=np.float32)
    wv = np.asarray(wv, dtype=np.float32)
    wo = np.asarray(wo, dtype=np.float32)
    pos = np.asarray(position_ids)

    # per-head rotate-half permutation: evens then odds
    base = np.concatenate([np.arange(0, HD, 2), np.arange(1, HD, 2)])
    perm = np.concatenate([h * HD + base for h in range(HLOC)])

    inv_freq = (
        1.0 / (np.float32(THETA) ** (np.arange(0, HD, 2, dtype=np.float32) / np.float32(HD)))
    ).astype(np.float32)

    in_maps = []
    for c in range(NCORES):
        b, hg = c // HG, c % HG
        rows = slice(hg * DLOC, (hg + 1) * DLOC)
        xT = _wire(x[b].T)
        wqT = _wire(wq[rows, :][perm].T)
        wkT = _wire(wk[rows, :][perm].T)
        wvT = _wire(wv[rows, :].T)
        woT = _wire(wo[:, rows].T)
        ang = (pos[b].astype(np.float32)[:, None] * inv_freq[None, :]).astype(np.float32)
        cos32 = np.cos(ang).astype(np.float32).T  # [32, T]
        sin32 = np.sin(ang).astype(np.float32).T
        cosT = _wire(np.tile(cos32, (4, 1)))
        # sign-baked sin rows: [+sin; -sin; +sin; -sin] (see RoPE in build_program)
        sinT = _wire(np.concatenate([sin32, -sin32, sin32, -sin32], axis=0))
        in_maps.append(
            {
                "xT": xT,
                "wqT": wqT,
                "wkT": wkT,
                "wvT": wvT,
                "woT": woT,
                "cosT": cosT,
                "sinT": sinT,
            }
        )
    return in_maps


def gather(results):
    """Sum the per-core partial outputs pairwise into the full output."""
    out = np.empty((B, T, D), dtype=np.float32)
    for b in range(B):
        out[b] = results[2 * b]["out"] + results[2 * b + 1]["out"]
    return out


_CACHED_NC = None


def kernel(x, wq, wk, wv, wo, position_ids):
    global _CACHED_NC
    if _CACHED_NC is None:
        _CACHED_NC = build_program()
    in_maps = prep_in_maps(x, wq, wk, wv, wo, position_ids)
    res = run_bass_kernel_spmd(_CACHED_NC, in_maps, list(range(NCORES)))
    return gather(res.results)
